# revision 2
# baseline (speedup 1.0000x reference)
"""Trainium2 Bass kernel for nn_BasicS2ConvV2.

Computes out[b,d,p,r] = sum_{c,k,a} W_eff[d,c,k,a,r] * x[b,c,k,p,a], where
W_eff[d,c,k,a,r] = W[d, c, M_idx[k,a,r]] is a pure index-gather of the small
parameter tensor W (materialized on the host).

Device strategy (per NeuronCore, x sharded over p into 8 slices of 1024):
  - x HBM layout [b, ck=416, p, a] (a innermost).  The only DMA-efficient
    load keeps (p, a) as the free dim: SBUF tiles [ck-chunk<=128, PT, 12],
    cast fp32->bf16 in the SWDGE DMA (24KB contiguous per partition row).
  - The einsum is a matmul with contraction (c,k,a)=4992.  We put ck-chunks
    on partitions and do the a-sum via 12 strided rhs views x[:, :, a]
    (free-dim stride 12), accumulating a and ck-chunks into one PSUM bank.
  - M packs (rsub, d) = 4 r's x 32 d's = 128 output partitions; 3 r-groups
    cover r=12.  Weights are host-packed bf16 lhsT tiles, resident in SBUF.
  - Output is written as out[b, rg, m=(rsub*32+d), p] (contiguous rows);
    the host transposes to [b, d, p, r] and concatenates the p-shards.
"""

import numpy as np
import ml_dtypes

# Problem shapes (hardcoded; harness runs kernel.py standalone).
B = 2
DIN = 32
DOUT = 32
KK = 13          # kernel size
A = 12           # anchor size
R = 12           # rotation copies
N_PARAM = 36
P_FULL = 8192
N_CORES = 8
P_LOC = P_FULL // N_CORES       # 1024 points per core
CK = DIN * KK                   # 416 contraction rows per a
N_CHUNKS = 4                    # ck chunks of 128,128,128,32 partitions
CHUNK_ROWS = [128, 128, 128, CK - 3 * 128]
PT = 512                        # p tile (PSUM bank = 512 fp32)
RG = 3                          # r groups (4 r's each)
RSUB = 4

_NC_CACHE = None


def _build_nc():
    import concourse.bacc as bacc
    import concourse.mybir as mybir
    import concourse.tile as tile

    nc = bacc.Bacc("TRN2", target_bir_lowering=False, debug=False,
                   num_devices=N_CORES)
    x_in = nc.dram_tensor("x", [B, CK, P_LOC, A], mybir.dt.float32,
                          kind="ExternalInput")
    wef_in = nc.dram_tensor("wef", [128, N_CHUNKS, A, RG, 128],
                            mybir.dt.bfloat16, kind="ExternalInput")
    out_t = nc.dram_tensor("out", [B, RG, 128, P_LOC], mybir.dt.float32,
                           kind="ExternalOutput")

    NPT = P_LOC // PT

    with tile.TileContext(nc) as tc:
        with (
            tc.tile_pool(name="wpool", bufs=1) as wpool,
            tc.tile_pool(name="xpool", bufs=2) as xpool,
            tc.tile_pool(name="opool", bufs=3) as opool,
            tc.tile_pool(name="pspool", bufs=4, space="PSUM") as pspool,
        ):
            W_sb = wpool.tile([128, N_CHUNKS, A, RG, 128], mybir.dt.bfloat16)
            nc.sync.dma_start(W_sb[:], wef_in[:])

            for pt in range(NPT):
                for b in range(B):
                    xt = []
                    for ch in range(N_CHUNKS):
                        rows = CHUNK_ROWS[ch]
                        t = xpool.tile([rows, PT, A], mybir.dt.bfloat16,
                                       tag=f"x{ch}")
                        nc.gpsimd.dma_start(
                            t[:],
                            x_in[b, ch * 128:ch * 128 + rows,
                                 pt * PT:(pt + 1) * PT, :],
                        )
                        xt.append(t)
                    for rg in range(RG):
                        ps = pspool.tile([128, PT], mybir.dt.float32, tag="ps")
                        n_mm = A * N_CHUNKS
                        i = 0
                        for a in range(A):
                            for ch in range(N_CHUNKS):
                                rows = CHUNK_ROWS[ch]
                                nc.tensor.matmul(
                                    ps[:, :],
                                    W_sb[0:rows, ch, a, rg, :],
                                    xt[ch][:, :, a],
                                    start=(i == 0),
                                    stop=(i == n_mm - 1),
                                )
                                i += 1
                        ot = opool.tile([128, PT], mybir.dt.float32, tag="ot")
                        nc.vector.tensor_copy(ot[:], ps[:])
                        nc.sync.dma_start(
                            out_t[b, rg, :, pt * PT:(pt + 1) * PT], ot[:])

    nc.compile()
    return nc


def _get_nc():
    global _NC_CACHE
    if _NC_CACHE is None:
        _NC_CACHE = _build_nc()
    return _NC_CACHE


def _host_weights(W, idx_map, idxs_k, idxs_a):
    """Build bf16 lhsT pack wef[q, chunk, a, rg, m=(rsub*32+d)]."""
    W = np.asarray(W, dtype=np.float32)
    idx_map = np.asarray(idx_map).astype(np.int64)
    idxs_k = np.asarray(idxs_k).astype(np.int64)
    idxs_a = np.asarray(idxs_a).astype(np.int64)

    Wr = W[:, :, idx_map].reshape(DOUT, DIN, KK, A)          # [d,c,k,a]
    a2 = idxs_a                                              # [K,A,R]
    k_ix = np.arange(KK)[:, None, None]
    r_ix = np.arange(R)[None, None, :]
    k2 = idxs_k[k_ix, a2, r_ix]                              # [K,A,R]
    W_eff = Wr[:, :, k2, a2]                                 # [d,c,K,A,R]

    # -> [ck, a, r, d] with ck = c*13 + k (matches x reshape [b,(c k),p,a])
    Wf = np.ascontiguousarray(W_eff.transpose(1, 2, 3, 4, 0)).reshape(
        CK, A, R, DOUT)
    # m = rsub*32 + d ; r = rg*4 + rsub
    Wf = Wf.reshape(CK, A, RG, RSUB * DOUT)                  # [ck,a,rg,m]
    wef_full = np.zeros((N_CHUNKS * 128, A, RG, RSUB * DOUT), np.float32)
    wef_full[:CK] = Wf
    wef = wef_full.reshape(N_CHUNKS, 128, A, RG, 128).transpose(1, 0, 2, 3, 4)
    return np.ascontiguousarray(wef).astype(ml_dtypes.bfloat16)


def _prepare_in_maps(inputs):
    x = np.asarray(inputs["x"], dtype=np.float32)
    wef = _host_weights(inputs["W"], inputs["idx_map"],
                        inputs["idxs_k"], inputs["idxs_a"])

    xr = x.reshape(B, CK, P_FULL, A)
    in_maps = []
    for core in range(N_CORES):
        xs = np.ascontiguousarray(
            xr[:, :, core * P_LOC:(core + 1) * P_LOC, :])
        in_maps.append({"x": xs, "wef": wef})
    return in_maps


def _decode_out(core_outs):
    """core_outs: list of per-core 'out' arrays [B,RG,128,P_LOC] -> full out."""
    shards = []
    for od in core_outs:
        od = od.reshape(B, RG, RSUB, DOUT, P_LOC)
        od = od.transpose(0, 3, 4, 1, 2).reshape(B, DOUT, P_LOC, R)
        shards.append(od)
    return np.ascontiguousarray(np.concatenate(shards, axis=2))


def _run(inputs, trace=False):
    from concourse.bass_utils import run_bass_kernel_spmd

    in_maps = _prepare_in_maps(inputs)
    nc = _get_nc()
    res = run_bass_kernel_spmd(nc, in_maps, core_ids=list(range(N_CORES)),
                               trace=trace)
    out = _decode_out([res.results[c]["out"] for c in range(N_CORES)])
    return out, res


def kernel(**inputs):
    out, _ = _run(inputs, trace=False)
    return out


# revision 4
# speedup vs baseline: 26.8223x; 26.8223x over previous
"""Trainium2 Bass kernel for nn_BasicS2ConvV2.

Computes out[b,d,p,r] = sum_{c,k,a} W_eff[d,c,k,a,r] * x[b,c,k,p,a], where
W_eff[d,c,k,a,r] = W[d, c, M_idx[k,a,r]] is a pure index-gather of the small
parameter tensor W (materialized on the host).

Device strategy (per NeuronCore, x sharded over p into 8 slices of 1024):
  - x HBM layout [b, ck=416, p, a] (a innermost).  The only DMA-efficient
    load keeps (p, a) as the free dim: SBUF tiles [ck-chunk<=128, PT, 12],
    cast fp32->bf16 in the SWDGE DMA (24KB contiguous per partition row).
  - The einsum is a matmul with contraction (c,k,a)=4992.  ck-chunks of 128
    go on partitions; the a-sum uses 12 strided rhs views x[:, :, a]
    (free-dim stride 12), accumulating a and ck-chunks into one PSUM bank.
  - The ck remainder (rows 384:416, only 32 wide) is repacked on the vector
    engine: 4 a-slices are partition-shift-copied into one [128, PT] tile,
    so all matmuls contract a full K=128 (39 matmuls per output tile
    instead of 48).
  - M packs (rsub, d) = 4 r's x 32 d's = 128 output partitions; 3 r-groups
    cover r=12.  Weights are host-packed bf16 lhsT tiles, resident in SBUF,
    loaded per-r-group so the first matmuls start early.
  - Output is written as out[b, rg, m=(rsub*32+d), p] (contiguous rows);
    the host transposes to [b, d, p, r] and concatenates the p-shards.
"""

import numpy as np
import ml_dtypes

# Problem shapes (hardcoded; harness runs kernel.py standalone).
B = 2
DIN = 32
DOUT = 32
KK = 13          # kernel size
A = 12           # anchor size
R = 12           # rotation copies
N_PARAM = 36
P_FULL = 8192
N_CORES = 8
P_LOC = P_FULL // N_CORES       # 1024 points per core
CK = DIN * KK                   # 416 contraction rows per a
PT = 512                        # p tile (PSUM bank = 512 fp32)
RG = 3                          # r groups (4 r's each)
RSUB = 4
NT = 39                         # lhsT tiles per r-group: 12a x 3ch + 3 packed

_NC_CACHE = None


def _build_nc(pt=PT, repeat=1):
    import concourse.bacc as bacc
    import concourse.mybir as mybir
    import concourse.tile as tile

    nc = bacc.Bacc("TRN2", target_bir_lowering=False, debug=False,
                   num_devices=N_CORES)
    x_in = nc.dram_tensor("x", [B, CK, P_LOC, A], mybir.dt.float32,
                          kind="ExternalInput")
    wef_in = nc.dram_tensor("wef", [128, RG, NT, 128],
                            mybir.dt.bfloat16, kind="ExternalInput")
    out_t = nc.dram_tensor("out", [B, RG, 128, P_LOC], mybir.dt.float32,
                           kind="ExternalOutput")

    npt = P_LOC // pt

    with tile.TileContext(nc) as tc:
        with (
            tc.tile_pool(name="wpool", bufs=1) as wpool,
            tc.tile_pool(name="xpool", bufs=2) as xpool,
            tc.tile_pool(name="opool", bufs=3) as opool,
            tc.tile_pool(name="pspool", bufs=4, space="PSUM") as pspool,
        ):
          for _rep in range(repeat):
            W_sb = wpool.tile([128, RG, NT, 128], mybir.dt.bfloat16,
                              tag="wsb")
            nc.sync.dma_start(W_sb[:, 0], wef_in[:, 0])

            first = True
            for pt_i in range(npt):
                for b in range(B):
                    xt = []
                    for ch in range(3):
                        t = xpool.tile([128, pt, A], mybir.dt.bfloat16,
                                       tag=f"x{ch}")
                        nc.gpsimd.dma_start(
                            t[:],
                            x_in[b, ch * 128:(ch + 1) * 128,
                                 pt_i * pt:(pt_i + 1) * pt, :])
                        xt.append(t)
                    x3 = xpool.tile([32, pt, A], mybir.dt.bfloat16, tag="x3")
                    nc.gpsimd.dma_start(
                        x3[:], x_in[b, 384:416, pt_i * pt:(pt_i + 1) * pt, :])
                    if first:
                        # remaining weight groups load behind the first x set
                        for rg in range(1, RG):
                            nc.sync.dma_start(W_sb[:, rg], wef_in[:, rg])
                        first = False
                    # pack ck-remainder: 4 a-slices -> one K=128 tile
                    pk = []
                    for j in range(3):
                        tp = xpool.tile([128, pt], mybir.dt.bfloat16,
                                        tag=f"pk{j}")
                        for g in range(4):
                            nc.vector.tensor_copy(
                                tp[32 * g:32 * (g + 1), :],
                                x3[:, :, 4 * j + g])
                        pk.append(tp)
                    for rg in range(RG):
                        ps = pspool.tile([128, pt], mybir.dt.float32,
                                         tag="ps")
                        i = 0
                        for a in range(A):
                            for ch in range(3):
                                nc.tensor.matmul(
                                    ps[:, :],
                                    W_sb[:, rg, a * 3 + ch, :],
                                    xt[ch][:, :, a],
                                    start=(i == 0), stop=False)
                                i += 1
                        for j in range(3):
                            nc.tensor.matmul(
                                ps[:, :],
                                W_sb[:, rg, 36 + j, :],
                                pk[j][:, :],
                                start=False, stop=(j == 2))
                        ot = opool.tile([128, pt], mybir.dt.float32,
                                        tag="ot")
                        nc.vector.tensor_copy(ot[:], ps[:])
                        nc.sync.dma_start(
                            out_t[b, rg, :, pt_i * pt:(pt_i + 1) * pt],
                            ot[:])

    nc.compile()
    return nc


def _get_nc():
    global _NC_CACHE
    if _NC_CACHE is None:
        _NC_CACHE = _build_nc()
    return _NC_CACHE


def _host_weights(W, idx_map, idxs_k, idxs_a):
    """Build bf16 lhsT pack wef[q, rg, t, m=(rsub*32+d)].

    Tiles t per r-group: t = a*3+ch (ch<3, rows q = ck=ch*128+q) for the
    full ck chunks; t = 36+j for the packed remainder, whose row q = 32g+qq
    holds ck = 384+qq at a = 4j+g.
    """
    W = np.asarray(W, dtype=np.float32)
    idx_map = np.asarray(idx_map).astype(np.int64)
    idxs_k = np.asarray(idxs_k).astype(np.int64)
    idxs_a = np.asarray(idxs_a).astype(np.int64)

    Wr = W[:, :, idx_map].reshape(DOUT, DIN, KK, A)          # [d,c,k,a]
    a2 = idxs_a                                              # [K,A,R]
    k_ix = np.arange(KK)[:, None, None]
    r_ix = np.arange(R)[None, None, :]
    k2 = idxs_k[k_ix, a2, r_ix]                              # [K,A,R]
    W_eff = Wr[:, :, k2, a2]                                 # [d,c,K,A,R]

    # -> [ck, a, rg, m] with ck = c*13 + k, m = rsub*32 + d, r = rg*4+rsub
    Wf = np.ascontiguousarray(W_eff.transpose(1, 2, 3, 4, 0)).reshape(
        CK, A, R, DOUT).reshape(CK, A, RG, RSUB * DOUT)

    wefA = Wf[:384].reshape(3, 128, A, RG, 128)              # [ch,q,a,rg,m]
    wefA = wefA.transpose(1, 3, 2, 0, 4).reshape(128, RG, 36, 128)

    wefB = Wf[384:].reshape(32, 3, 4, RG, 128)               # [qq,j,g,rg,m]
    wefB = wefB.transpose(2, 0, 3, 1, 4).reshape(128, RG, 3, 128)

    wef = np.concatenate([wefA, wefB], axis=2)               # [128,RG,39,128]
    return np.ascontiguousarray(wef).astype(ml_dtypes.bfloat16)


def _prepare_in_maps(inputs):
    x = np.asarray(inputs["x"], dtype=np.float32)
    wef = _host_weights(inputs["W"], inputs["idx_map"],
                        inputs["idxs_k"], inputs["idxs_a"])

    xr = x.reshape(B, CK, P_FULL, A)
    in_maps = []
    for core in range(N_CORES):
        xs = np.ascontiguousarray(
            xr[:, :, core * P_LOC:(core + 1) * P_LOC, :])
        in_maps.append({"x": xs, "wef": wef})
    return in_maps


def _decode_out(core_outs):
    """core_outs: list of per-core 'out' arrays [B,RG,128,P_LOC] -> full."""
    shards = []
    for od in core_outs:
        od = np.asarray(od).reshape(B, RG, RSUB, DOUT, P_LOC)
        od = od.transpose(0, 3, 4, 1, 2).reshape(B, DOUT, P_LOC, R)
        shards.append(od)
    return np.ascontiguousarray(np.concatenate(shards, axis=2))


def _run(inputs, trace=False):
    from concourse.bass_utils import run_bass_kernel_spmd

    in_maps = _prepare_in_maps(inputs)
    nc = _get_nc()
    res = run_bass_kernel_spmd(nc, in_maps, core_ids=list(range(N_CORES)),
                               trace=trace)
    out = _decode_out([res.results[c]["out"] for c in range(N_CORES)])
    return out, res


def kernel(**inputs):
    out, _ = _run(inputs, trace=False)
    return out


# revision 12
# speedup vs baseline: 40.6637x; 1.5160x over previous
"""Trainium2 Bass kernel for nn_BasicS2ConvV2.

Computes out[b,d,p,r] = sum_{c,k,a} W_eff[d,c,k,a,r] * x[b,c,k,p,a], where
W_eff[d,c,k,a,r] = W[d, c, M_idx[k,a,r]] is a pure index-gather of the small
parameter tensor W (materialized on the host).

Device strategy (per NeuronCore, x sharded over p into 8 slices of 1024):
  - x HBM layout [b, ck=416, p, a] (a innermost).  The only DMA-efficient
    load keeps (p, a) as the free dim: SBUF tiles [ck-chunk<=128, PT, 12],
    cast fp32->bf16 in the SWDGE DMA (24KB contiguous per partition row).
  - The einsum is a matmul with contraction (c,k,a)=4992.  ck-chunks of 128
    go on partitions; the a-sum uses 12 strided rhs views x[:, :, a]
    (free-dim stride 12), accumulating a and ck-chunks into one PSUM bank.
  - The ck remainder (rows 384:416, only 32 wide) is repacked on the vector
    engine: 4 a-slices are partition-shift-copied into one [128, PT] tile,
    so all matmuls contract a full K=128 (39 matmuls per output tile
    instead of 48).
  - M packs (rsub, d) = 4 r's x 32 d's = 128 output partitions; 3 r-groups
    cover r=12.  Weights are host-packed bf16 lhsT tiles, resident in SBUF,
    loaded per-r-group so the first matmuls start early.
  - Output is written as out[b, rg, m=(rsub*32+d), p] (contiguous rows);
    the host transposes to [b, d, p, r] and concatenates the p-shards.
"""

import numpy as np
import ml_dtypes

# Problem shapes (hardcoded; harness runs kernel.py standalone).
B = 2
DIN = 32
DOUT = 32
KK = 13          # kernel size
A = 12           # anchor size
R = 12           # rotation copies
N_PARAM = 36
P_FULL = 8192
N_CORES = 8
P_LOC = P_FULL // N_CORES       # 1024 points per core
CK = DIN * KK                   # 416 contraction rows per a
PT = 256                        # p tile (<= 512 fp32 PSUM bank)
RG = 3                          # r groups (4 r's each)
RSUB = 4
NT = 39                         # lhsT tiles per r-group: 12a x 3ch + 3 packed

_NC_CACHE = None


def _build_nc(pt=PT, repeat=1, xdt="bf16", dense=True, x_bufs=2):
    import concourse.bacc as bacc
    import concourse.mybir as mybir
    import concourse.tile as tile

    # compute dtype: bf16 (SWDGE cast during x load) or float32r (raw fp32
    # bits, PE "replicated fp32" mode — no cast, HWDGE loads)
    cdt = mybir.dt.bfloat16 if xdt == "bf16" else mybir.dt.float32r
    xdt_in = mybir.dt.float32 if xdt == "bf16" else mybir.dt.float32r
    x_eng = "gpsimd" if xdt == "bf16" else "sync"

    nc = bacc.Bacc("TRN2", target_bir_lowering=False, debug=False,
                   num_devices=N_CORES)
    x_in = nc.dram_tensor("x", [B, CK, P_LOC, A], xdt_in,
                          kind="ExternalInput")
    wef_in = nc.dram_tensor("wef", [128, RG, NT, 128],
                            cdt, kind="ExternalInput")
    out_t = nc.dram_tensor("out", [B, RG, 128, P_LOC], mybir.dt.float32,
                           kind="ExternalOutput")

    npt = P_LOC // pt

    with tile.TileContext(nc) as tc:
        with (
            tc.tile_pool(name="wpool", bufs=1) as wpool,
            tc.tile_pool(name="xpool", bufs=x_bufs) as xpool,
            tc.tile_pool(name="opool", bufs=3) as opool,
            tc.tile_pool(name="pspool", bufs=4, space="PSUM") as pspool,
        ):
          for _rep in range(repeat):
            W_sb = wpool.tile([128, RG, NT, 128], cdt, tag="wsb")
            nc.sync.dma_start(W_sb[:, 0], wef_in[:, 0])

            first = True
            for pt_i in range(npt):
                for b in range(B):
                    xt = []
                    for ch in range(3):
                        t = xpool.tile([128, pt, A], cdt,
                                       tag=f"x{ch}")
                        getattr(nc, x_eng).dma_start(
                            t[:],
                            x_in[b, ch * 128:(ch + 1) * 128,
                                 pt_i * pt:(pt_i + 1) * pt, :])
                        xt.append(t)
                    x3 = xpool.tile([32, pt, A], cdt, tag="x3")
                    getattr(nc, x_eng).dma_start(
                        x3[:], x_in[b, 384:416, pt_i * pt:(pt_i + 1) * pt, :])
                    if first:
                        # remaining weight groups load behind the first x set
                        for rg in range(1, RG):
                            nc.sync.dma_start(W_sb[:, rg], wef_in[:, rg])
                        first = False
                    # de-stride: copy each a-slice into a dense [128, pt]
                    # rhs tile (strided rhs slows the PE stream ~5x on HW).
                    # Engine-agnostic copies let Tile balance DVE/ACT.
                    dt_map = {}
                    if dense:
                        for a in range(A):
                            for ch in range(3):
                                d_t = xpool.tile([128, pt], cdt,
                                                 tag=f"d{a}_{ch}", bufs=1)
                                nc.any.tensor_copy(d_t[:], xt[ch][:, :, a])
                                dt_map[(a, ch)] = d_t
                    # pack ck-remainder: 4 a-slices -> one K=128 tile
                    pk = []
                    for j in range(3):
                        tp = xpool.tile([128, pt], cdt,
                                        tag=f"pk{j}")
                        for g in range(4):
                            nc.vector.tensor_copy(
                                tp[32 * g:32 * (g + 1), :],
                                x3[:, :, 4 * j + g])
                        pk.append(tp)
                    for rg in range(RG):
                        ps = pspool.tile([128, pt], mybir.dt.float32,
                                         tag="ps")
                        i = 0
                        for a in range(A):
                            for ch in range(3):
                                rhs = (dt_map[(a, ch)][:, :] if dense
                                       else xt[ch][:, :, a])
                                nc.tensor.matmul(
                                    ps[:, :],
                                    W_sb[:, rg, a * 3 + ch, :],
                                    rhs,
                                    start=(i == 0), stop=False)
                                i += 1
                        for j in range(3):
                            nc.tensor.matmul(
                                ps[:, :],
                                W_sb[:, rg, 36 + j, :],
                                pk[j][:, :],
                                start=False, stop=(j == 2))
                        ot = opool.tile([128, pt], mybir.dt.float32,
                                        tag="ot")
                        nc.vector.tensor_copy(ot[:], ps[:])
                        nc.sync.dma_start(
                            out_t[b, rg, :, pt_i * pt:(pt_i + 1) * pt],
                            ot[:])

    nc.compile()
    return nc


def _get_nc():
    global _NC_CACHE
    if _NC_CACHE is None:
        _NC_CACHE = _build_nc()
    return _NC_CACHE


def _host_weights(W, idx_map, idxs_k, idxs_a, xdt="bf16"):
    """Build bf16 lhsT pack wef[q, rg, t, m=(rsub*32+d)].

    Tiles t per r-group: t = a*3+ch (ch<3, rows q = ck=ch*128+q) for the
    full ck chunks; t = 36+j for the packed remainder, whose row q = 32g+qq
    holds ck = 384+qq at a = 4j+g.
    """
    W = np.asarray(W, dtype=np.float32)
    idx_map = np.asarray(idx_map).astype(np.int64)
    idxs_k = np.asarray(idxs_k).astype(np.int64)
    idxs_a = np.asarray(idxs_a).astype(np.int64)

    Wr = W[:, :, idx_map].reshape(DOUT, DIN, KK, A)          # [d,c,k,a]
    a2 = idxs_a                                              # [K,A,R]
    k_ix = np.arange(KK)[:, None, None]
    r_ix = np.arange(R)[None, None, :]
    k2 = idxs_k[k_ix, a2, r_ix]                              # [K,A,R]
    W_eff = Wr[:, :, k2, a2]                                 # [d,c,K,A,R]

    # -> [ck, a, rg, m] with ck = c*13 + k, m = rsub*32 + d, r = rg*4+rsub
    Wf = np.ascontiguousarray(W_eff.transpose(1, 2, 3, 4, 0)).reshape(
        CK, A, R, DOUT).reshape(CK, A, RG, RSUB * DOUT)

    wefA = Wf[:384].reshape(3, 128, A, RG, 128)              # [ch,q,a,rg,m]
    wefA = wefA.transpose(1, 3, 2, 0, 4).reshape(128, RG, 36, 128)

    wefB = Wf[384:].reshape(32, 3, 4, RG, 128)               # [qq,j,g,rg,m]
    wefB = wefB.transpose(2, 0, 3, 1, 4).reshape(128, RG, 3, 128)

    wef = np.concatenate([wefA, wefB], axis=2)               # [128,RG,39,128]
    odt = ml_dtypes.bfloat16 if xdt == "bf16" else np.float32
    return np.ascontiguousarray(wef).astype(odt)


def _prepare_in_maps(inputs, xdt="bf16"):
    x = np.asarray(inputs["x"], dtype=np.float32)
    wef = _host_weights(inputs["W"], inputs["idx_map"],
                        inputs["idxs_k"], inputs["idxs_a"], xdt=xdt)

    xr = x.reshape(B, CK, P_FULL, A)
    in_maps = []
    for core in range(N_CORES):
        xs = np.ascontiguousarray(
            xr[:, :, core * P_LOC:(core + 1) * P_LOC, :])
        in_maps.append({"x": xs, "wef": wef})
    return in_maps


def _decode_out(core_outs):
    """core_outs: list of per-core 'out' arrays [B,RG,128,P_LOC] -> full."""
    shards = []
    for od in core_outs:
        od = np.asarray(od).reshape(B, RG, RSUB, DOUT, P_LOC)
        od = od.transpose(0, 3, 4, 1, 2).reshape(B, DOUT, P_LOC, R)
        shards.append(od)
    return np.ascontiguousarray(np.concatenate(shards, axis=2))


def _run(inputs, trace=False):
    from concourse.bass_utils import run_bass_kernel_spmd

    in_maps = _prepare_in_maps(inputs)
    nc = _get_nc()
    res = run_bass_kernel_spmd(nc, in_maps, core_ids=list(range(N_CORES)),
                               trace=trace)
    out = _decode_out([res.results[c]["out"] for c in range(N_CORES)])
    return out, res


def kernel(**inputs):
    out, _ = _run(inputs, trace=False)
    return out
